# revision 2
# baseline (speedup 1.0000x reference)
"""Multi-head latent attention (MLA) on Trainium2 — 8-core SPMD Bass kernel.

Reference computation (fp32):
    Q  = X @ W_q.T           [B,S,1024] -> heads [B,H,S,256]
    Kc = X @ W_k.T           [B,S,256]  (shared across heads, MQA-style)
    Vc = X @ W_v.T           [B,S,256]
    P  = softmax(Q Kc^T / sqrt(256))
    Y  = concat_h(P Vc) @ W_o.T

Sharding: 8 cores = (batch b in {0,1}) x (query s-chunk in {0..3}).
Each core gets the full X_b^T (for K/V projections, recomputed per core) plus
its own 1024-query column slice, computes attention for all 4 heads over its
queries, and writes its [1024, 1024] fp32 output slice.  Host concatenates.

All matmuls run in bf16 with fp32 PSUM accumulation; softmax runs in fp32 on
the scalar (ACT) engine.  Scores are computed transposed (keys on partitions)
so softmax-normalisation is deferred: Y~ = exp(S^T) matmuls are unnormalised,
and each head's output is scaled by 1/l broadcast before the W_o projection.
Row sums l are accumulated on the (otherwise idle) vector engine.

Measured numerics (host emulation vs fp32 reference): rel-fro err ~4e-3.
"""

import numpy as np
import ml_dtypes
from contextlib import ExitStack

import concourse.bass as bass
import concourse.tile as tile
from concourse import bacc, mybir
from concourse.bass_utils import run_bass_kernel_spmd

# ---- problem constants (hardcoded; kernel.py must be self-contained) ----
B, S, DM = 2, 4096, 1024
H, DK, DKV = 4, 256, 256
NCORES = 8
CHUNKS = 4                # query chunks per batch
SQ = S // CHUNKS          # 1024 queries per core
SCALE = 1.0 / 16.0        # 1/sqrt(DK)

P = 128                   # partitions
NT = S // P               # 32 key tiles
NCT = DM // P             # 8 contraction tiles over the model dim
FD = 512                  # matmul moving free-dim chunk (one fp32 PSUM bank)
NSH = SQ // FD            # 2 query free-dim chunks

BF16 = mybir.dt.bfloat16
F32 = mybir.dt.float32
NPBF16 = ml_dtypes.bfloat16


def _emit_full(tc: tile.TileContext, y, xt, xq, wqt, wkt, wvt, wot):
    """Emit the complete per-core program (projections + attention + W_o)."""
    nc = tc.nc
    AF = mybir.ActivationFunctionType

    with ExitStack() as ctx:
        acts = ctx.enter_context(tc.tile_pool(name="acts", bufs=1))

        ps_sc = ctx.enter_context(tc.tile_pool(name="ps_sc", bufs=2, space="PSUM"))
        ps_ov = ctx.enter_context(tc.tile_pool(name="ps_ov", bufs=4, space="PSUM"))
        ps_lrl = ctx.enter_context(tc.tile_pool(name="ps_lrl", bufs=1, space="PSUM"))

        qt_sb = [acts.tile([P, SQ], BF16, tag=f"qt{j}", name=f"qt_sb{j}") for j in range(NCT)]
        kt_sb = [acts.tile([P, S], BF16, tag=f"kt{j}", name=f"kt_sb{j}") for j in range(2)]
        vc_sb = [acts.tile([P, DKV], BF16, tag=f"vc{t}", name=f"vc_sb{t}") for t in range(NT)]
        ot_sb = [acts.tile([P, SQ], BF16, tag=f"ot{j}", name=f"ot_sb{j}") for j in range(NCT)]
        ones_col = acts.tile([P, 1], F32, tag="ones_col", name="ones_col")
        ones_row = acts.tile([1, P], F32, tag="ones_row", name="ones_row")
        nc.vector.memset(ones_col, 1.0)
        nc.vector.memset(ones_row, 1.0)

        # ---- phase P (projection inputs live only inside this block) ----
        with tc.tile_pool(name="loadin", bufs=1) as loadin:
            xq_sb = [loadin.tile([P, SQ], BF16, tag=f"xq{i}", name=f"xq_sb{i}") for i in range(NCT)]
            wqt_sb = [loadin.tile([P, DM], BF16, tag=f"wq{i}", name=f"wqt_sb{i}") for i in range(NCT)]
            xt_sb = [loadin.tile([P, S], BF16, tag=f"xt{i}", name=f"xt_sb{i}") for i in range(NCT)]
            wkt_sb = [loadin.tile([P, DKV], BF16, tag=f"wk{i}", name=f"wkt_sb{i}") for i in range(NCT)]
            wvt_sb = [loadin.tile([P, DKV], BF16, tag=f"wv{i}", name=f"wvt_sb{i}") for i in range(NCT)]
            for i in range(NCT):
                nc.sync.dma_start(out=xq_sb[i], in_=xq[i * P:(i + 1) * P, :])
                nc.sync.dma_start(out=wqt_sb[i], in_=wqt[i * P:(i + 1) * P, :])
            for i in range(NCT):
                nc.sync.dma_start(out=xt_sb[i], in_=xt[i * P:(i + 1) * P, :])
                nc.sync.dma_start(out=wkt_sb[i], in_=wkt[i * P:(i + 1) * P, :])
                nc.sync.dma_start(out=wvt_sb[i], in_=wvt[i * P:(i + 1) * P, :])

            for j in range(NCT):
                for sh in range(NSH):
                    ps = ps_sc.tile([P, FD], F32, tag="sc", name="ps_qt")
                    for i in range(NCT):
                        nc.tensor.matmul(
                            ps, wqt_sb[i][:, j * P:(j + 1) * P],
                            xq_sb[i][:, sh * FD:(sh + 1) * FD],
                            start=(i == 0), stop=(i == NCT - 1))
                    nc.scalar.activation(qt_sb[j][:, sh * FD:(sh + 1) * FD], ps, AF.Copy)

            for j in range(2):
                for tch in range(S // FD):
                    ps = ps_sc.tile([P, FD], F32, tag="sc", name="ps_kt")
                    for i in range(NCT):
                        nc.tensor.matmul(
                            ps, wkt_sb[i][:, j * P:(j + 1) * P],
                            xt_sb[i][:, tch * FD:(tch + 1) * FD],
                            start=(i == 0), stop=(i == NCT - 1))
                    nc.scalar.activation(kt_sb[j][:, tch * FD:(tch + 1) * FD], ps, AF.Copy)

            for t in range(NT):
                ps = ps_sc.tile([P, DKV], F32, tag="sc", name="ps_vc")
                for i in range(NCT):
                    nc.tensor.matmul(
                        ps, xt_sb[i][:, t * P:(t + 1) * P], wvt_sb[i],
                        start=(i == 0), stop=(i == NCT - 1))
                nc.scalar.activation(vc_sb[t], ps, AF.Copy)

        # ---- attention phase (new pools reuse loadin's SBUF) ----
        attp = ctx.enter_context(tc.tile_pool(name="attp", bufs=1))
        pt_pool = ctx.enter_context(tc.tile_pool(name="pt", bufs=NT))
        lpool = ctx.enter_context(tc.tile_pool(name="lpool", bufs=2))
        ypool = ctx.enter_context(tc.tile_pool(name="ypool", bufs=2))

        wot_sb = [attp.tile([P, DM], BF16, tag=f"wo{j}", name=f"wot_sb{j}") for j in range(NCT)]
        for j in range(NCT):
            nc.sync.dma_start(out=wot_sb[j], in_=wot[j * P:(j + 1) * P, :])

        for h in range(H):
            # scores^T + exp + row-sum partials
            lpart = lpool.tile([P, SQ], F32, tag="lp", name="lpart")
            pt_tiles = []
            for t in range(NT):
                ptt = pt_pool.tile([P, SQ], BF16, tag="pt", name="pt_t")
                pt_tiles.append(ptt)
                for sh in range(NSH):
                    ps = ps_sc.tile([P, FD], F32, tag="sc", name="ps_s")
                    nc.tensor.matmul(
                        ps, kt_sb[0][:, t * P:(t + 1) * P],
                        qt_sb[2 * h][:, sh * FD:(sh + 1) * FD],
                        start=True, stop=False)
                    nc.tensor.matmul(
                        ps, kt_sb[1][:, t * P:(t + 1) * P],
                        qt_sb[2 * h + 1][:, sh * FD:(sh + 1) * FD],
                        start=False, stop=True)
                    nc.scalar.activation(
                        ptt[:, sh * FD:(sh + 1) * FD], ps, AF.Exp, scale=SCALE)
                if t == 0:
                    nc.vector.tensor_copy(lpart, ptt)
                else:
                    nc.vector.tensor_add(lpart, lpart, ptt)

            # unnormalised attention output: O~^T[d, s] += Vc[t,d]^T P^T[t,s]
            ov_pairs = []
            for sh in range(NSH):
                ov0 = ps_ov.tile([P, FD], F32, tag="ov", name="ps_ov0")
                ov1 = ps_ov.tile([P, FD], F32, tag="ov", name="ps_ov1")
                ov_pairs.append((ov0, ov1))
                for t in range(NT):
                    nc.tensor.matmul(
                        ov0, vc_sb[t][:, 0:P],
                        pt_tiles[t][:, sh * FD:(sh + 1) * FD],
                        start=(t == 0), stop=(t == NT - 1))
                    nc.tensor.matmul(
                        ov1, vc_sb[t][:, P:DKV],
                        pt_tiles[t][:, sh * FD:(sh + 1) * FD],
                        start=(t == 0), stop=(t == NT - 1))

            # l = sum_t P^T[t, s] (partition sum via ones matmul), rl = 1/l,
            # RL = broadcast of rl over 128 partitions (outer product).
            l_ps = ps_lrl.tile([1, SQ], F32, tag="lrl", name="ps_l")
            for sh in range(NSH):
                nc.tensor.matmul(
                    l_ps[:, sh * FD:(sh + 1) * FD], ones_col,
                    lpart[:, sh * FD:(sh + 1) * FD], start=True, stop=True)
            rl_row = lpool.tile([1, SQ], F32, tag="rl_row", name="rl_row")
            nc.vector.reciprocal(rl_row, l_ps)
            rl_ps = ps_lrl.tile([P, SQ], F32, tag="lrl", name="ps_rl")
            for sh in range(NSH):
                nc.tensor.matmul(
                    rl_ps[:, sh * FD:(sh + 1) * FD], ones_row,
                    rl_row[:, sh * FD:(sh + 1) * FD], start=True, stop=True)
            rlb = lpool.tile([P, SQ], F32, tag="rlb", name="rlb")
            nc.scalar.activation(rlb, rl_ps, AF.Copy)

            # normalise while copying PSUM -> SBUF (bf16 for the W_o matmul)
            for sh in range(NSH):
                ov0, ov1 = ov_pairs[sh]
                nc.vector.tensor_mul(
                    ot_sb[2 * h][:, sh * FD:(sh + 1) * FD], ov0,
                    rlb[:, sh * FD:(sh + 1) * FD])
                nc.vector.tensor_mul(
                    ot_sb[2 * h + 1][:, sh * FD:(sh + 1) * FD], ov1,
                    rlb[:, sh * FD:(sh + 1) * FD])

        # ---- phase W: Y = O @ W_o^T ----
        for sb in range(SQ // P):
            ysb = ypool.tile([P, DM], F32, tag="y", name="ysb")
            for ec in range(DM // FD):
                ps = ps_sc.tile([P, FD], F32, tag="sc", name="ps_y")
                for j in range(NCT):
                    nc.tensor.matmul(
                        ps, ot_sb[j][:, sb * P:(sb + 1) * P],
                        wot_sb[j][:, ec * FD:(ec + 1) * FD],
                        start=(j == 0), stop=(j == NCT - 1))
                nc.scalar.activation(ysb[:, ec * FD:(ec + 1) * FD], ps, AF.Copy)
            nc.sync.dma_start(out=y[sb * P:(sb + 1) * P, :], in_=ysb)


_BUILD_CACHE = {}


def build_program():
    """Build + compile the per-core Bass program (cached per process)."""
    if "nc" in _BUILD_CACHE:
        return _BUILD_CACHE["nc"]
    nc = bacc.Bacc("TRN2", target_bir_lowering=False, debug=False)
    xt = nc.dram_tensor("xt", [DM, S], BF16, kind="ExternalInput").ap()
    xq = nc.dram_tensor("xq", [DM, SQ], BF16, kind="ExternalInput").ap()
    wqt = nc.dram_tensor("wqt", [DM, DM], BF16, kind="ExternalInput").ap()
    wkt = nc.dram_tensor("wkt", [DM, DKV], BF16, kind="ExternalInput").ap()
    wvt = nc.dram_tensor("wvt", [DM, DKV], BF16, kind="ExternalInput").ap()
    wot = nc.dram_tensor("wot", [DM, DM], BF16, kind="ExternalInput").ap()
    y = nc.dram_tensor("y", [SQ, DM], F32, kind="ExternalOutput").ap()
    with tile.TileContext(nc) as tc:
        _emit_full(tc, y, xt, xq, wqt, wkt, wvt, wot)
    nc.compile()
    _BUILD_CACHE["nc"] = nc
    return nc


def make_in_maps(X, W_q, W_k, W_v, W_o):
    """Host-side shard prep: transpose + bf16-cast, one input dict per core."""
    wqt = np.ascontiguousarray(W_q.T).astype(NPBF16)
    wkt = np.ascontiguousarray(W_k.T).astype(NPBF16)
    wvt = np.ascontiguousarray(W_v.T).astype(NPBF16)
    wot = np.ascontiguousarray(W_o.T).astype(NPBF16)
    xts = [np.ascontiguousarray(X[b].T).astype(NPBF16) for b in range(B)]
    in_maps = []
    for c in range(NCORES):
        b, chunk = divmod(c, CHUNKS)
        xq = np.ascontiguousarray(xts[b][:, chunk * SQ:(chunk + 1) * SQ])
        in_maps.append({
            "xt": xts[b], "xq": xq,
            "wqt": wqt, "wkt": wkt, "wvt": wvt, "wot": wot,
        })
    return in_maps


def run(X, W_q, W_k, W_v, W_o, trace=False, trace_cores=None):
    """Run the 8-core kernel; returns (Y, BassKernelResults)."""
    nc = build_program()
    in_maps = make_in_maps(X, W_q, W_k, W_v, W_o)
    res = run_bass_kernel_spmd(
        nc, in_maps, list(range(NCORES)), trace=trace, trace_cores=trace_cores)
    Y = np.empty((B, S, DM), np.float32)
    for c in range(NCORES):
        b, chunk = divmod(c, CHUNKS)
        Y[b, chunk * SQ:(chunk + 1) * SQ, :] = res.results[c]["y"]
    return Y, res


def kernel(X, W_q, W_k, W_v, W_o):
    X = np.asarray(X)
    W_q = np.asarray(W_q)
    W_k = np.asarray(W_k)
    W_v = np.asarray(W_v)
    W_o = np.asarray(W_o)
    Y, _ = run(X, W_q, W_k, W_v, W_o)
    return Y


# revision 15
# speedup vs baseline: 1.2107x; 1.2107x over previous
"""Multi-head latent attention (MLA) on Trainium2 — 8-core SPMD Bass kernel.

Reference computation (fp32):
    Q  = X @ W_q.T           [B,S,1024] -> heads [B,H,S,256]
    Kc = X @ W_k.T           [B,S,256]  (shared across heads, MQA-style)
    Vc = X @ W_v.T           [B,S,256]
    P  = softmax(Q Kc^T / sqrt(256))
    Y  = concat_h(P Vc) @ W_o.T

Sharding: 8 cores = (batch b in {0,1}) x (query s-chunk in {0..3}).
Each core projects Q/Kc/Vc for its own 1024-token chunk, the compressed
Kc^T / Vc are AllGathered across the 4 cores of each batch (replica groups
[[0..3],[4..7]]), then each core runs attention for all 4 heads over its
queries and writes its [1024, 1024] fp32 output slice.  Host concatenates.

All matmuls run in bf16 with fp32 PSUM accumulation; softmax runs in fp32 on
the scalar (ACT) engine.  Scores are computed transposed (keys on partitions)
so softmax-normalisation is deferred: the P^T @ Vc matmuls are unnormalised
and each head's output is scaled by 1/l (broadcast via a rank-1 matmul)
before the W_o projection.  Row sums l accumulate on the idle vector engine.

Measured numerics (CoreSim + HW): rel-fro err ~4.3e-3 vs fp32 reference.
"""

import numpy as np
import ml_dtypes
from contextlib import ExitStack

import concourse.bass as bass
import concourse.tile as tile
from concourse import bacc, mybir
from concourse.bass_utils import run_bass_kernel_spmd

# ---- problem constants (hardcoded; kernel.py must be self-contained) ----
B, S, DM = 2, 4096, 1024
H, DK, DKV = 4, 256, 256
NCORES = 8
CHUNKS = 4                # query chunks per batch
SQ = S // CHUNKS          # 1024 queries per core
SCALE = 1.0 / 16.0        # 1/sqrt(DK)

P = 128                   # partitions
NT = S // P               # 32 key tiles
NCT = DM // P             # 8 contraction tiles over the model dim
FD = 512                  # matmul moving free-dim chunk (one fp32 PSUM bank)
NSH = SQ // FD            # 2 query free-dim chunks

BF16 = mybir.dt.bfloat16
F32 = mybir.dt.float32
NPBF16 = ml_dtypes.bfloat16

# Use on-device AllGather to share Kc/Vc across the 4 cores of a batch
# (False recomputes them from the full X_b on every core).  The cost model
# predicts the AllGather latency (~135us) exceeds the ~40us of duplicated
# projection work it saves, so the recompute path is the default.
USE_CC = False

# emission variants (A/B tuning knobs)
DMA_Q_FIRST = True
ATTNV_INTERLEAVE = False
PS_SC_BUFS = 4
PS_OV_BUFS = 3
PS_LRL_BUFS = 1
LRL_SPLIT = True
EXP_WIDE = False


def _emit_full(tc: tile.TileContext, nc_io, use_cc):
    """Emit the complete per-core program (projections + attention + W_o)."""
    nc = tc.nc
    AF = mybir.ActivationFunctionType
    y, xt, xq, wqt, wkt, wvt, wot = nc_io

    with ExitStack() as ctx:
        acts = ctx.enter_context(tc.tile_pool(name="acts", bufs=1))

        ps_sc = ctx.enter_context(tc.tile_pool(name="ps_sc", bufs=PS_SC_BUFS, space="PSUM"))
        ps_ov = ctx.enter_context(tc.tile_pool(name="ps_ov", bufs=PS_OV_BUFS, space="PSUM"))
        ps_lrl = ctx.enter_context(tc.tile_pool(name="ps_lrl", bufs=PS_LRL_BUFS, space="PSUM"))

        qt_sb = [acts.tile([P, SQ], BF16, tag=f"qt{j}", name=f"qt_sb{j}") for j in range(NCT)]
        kt_sb = [acts.tile([P, S], BF16, tag=f"kt{j}", name=f"kt_sb{j}") for j in range(2)]
        vc_sb = [acts.tile([P, DKV], BF16, tag=f"vc{t}", name=f"vc_sb{t}") for t in range(NT)]
        ot_sb = [acts.tile([P, SQ], BF16, tag=f"ot{j}", name=f"ot_sb{j}") for j in range(NCT)]
        ones_col = acts.tile([P, 1], F32, tag="ones_col", name="ones_col")
        ones_row = acts.tile([1, P], F32, tag="ones_row", name="ones_row")
        nc.vector.memset(ones_col, 1.0)
        nc.vector.memset(ones_row, 1.0)

        # ---- phase P (projection inputs live only inside this block) ----
        with tc.tile_pool(name="loadin", bufs=1) as loadin:
            xq_sb = [loadin.tile([P, SQ], BF16, tag=f"xq{i}", name=f"xq_sb{i}") for i in range(NCT)]
            wqt_sb = [loadin.tile([P, DM], BF16, tag=f"wq{i}", name=f"wqt_sb{i}") for i in range(NCT)]
            wkt_sb = [loadin.tile([P, DKV], BF16, tag=f"wk{i}", name=f"wkt_sb{i}") for i in range(NCT)]
            wvt_sb = [loadin.tile([P, DKV], BF16, tag=f"wv{i}", name=f"wvt_sb{i}") for i in range(NCT)]
            for i in range(NCT):
                nc.sync.dma_start(out=xq_sb[i], in_=xq[i * P:(i + 1) * P, :])
                nc.sync.dma_start(out=wqt_sb[i], in_=wqt[i * P:(i + 1) * P, :])
            if not use_cc:
                xt_sb = [loadin.tile([P, S], BF16, tag=f"xt{i}", name=f"xt_sb{i}")
                         for i in range(NCT)]
                for i in range(NCT):
                    nc.sync.dma_start(out=xt_sb[i], in_=xt[i * P:(i + 1) * P, :])
                    nc.sync.dma_start(out=wkt_sb[i], in_=wkt[i * P:(i + 1) * P, :])
                    nc.sync.dma_start(out=wvt_sb[i], in_=wvt[i * P:(i + 1) * P, :])
            else:
                for i in range(NCT):
                    nc.sync.dma_start(out=wkt_sb[i], in_=wkt[i * P:(i + 1) * P, :])
                    nc.sync.dma_start(out=wvt_sb[i], in_=wvt[i * P:(i + 1) * P, :])

            if use_cc:
                # -- K/V for the local chunk only, then AllGather over the batch --
                GROUPS = [[0, 1, 2, 3], [4, 5, 6, 7]]
                kc_slice = nc.dram_tensor("kc_slice", [DKV, SQ], BF16).ap()
                vc_slice = nc.dram_tensor("vc_slice", [SQ, DKV], BF16).ap()
                kc_ag = nc.dram_tensor("kc_ag", [CHUNKS, DKV, SQ], BF16).ap()
                vc_ag = nc.dram_tensor("vc_ag", [CHUNKS, SQ, DKV], BF16).ap()

                # Kc^T slice [DKV, SQ] from the local chunk columns (xq).
                for j in range(2):
                    ktloc = loadin.tile([P, SQ], BF16, tag=f"ktloc{j}", name=f"ktloc{j}")
                    for sh in range(NSH):
                        ps = ps_sc.tile([P, FD], F32, tag="sc", name="ps_kt")
                        for i in range(NCT):
                            nc.tensor.matmul(
                                ps, wkt_sb[i][:, j * P:(j + 1) * P],
                                xq_sb[i][:, sh * FD:(sh + 1) * FD],
                                start=(i == 0), stop=(i == NCT - 1))
                        nc.scalar.activation(ktloc[:, sh * FD:(sh + 1) * FD], ps, AF.Copy)
                    nc.sync.dma_start(out=kc_slice[j * P:(j + 1) * P, :], in_=ktloc)

                # Vc slice [SQ, DKV] from the local chunk.
                for tl in range(SQ // P):
                    vcloc = loadin.tile([P, DKV], BF16, tag="vcloc", name="vcloc", bufs=4)
                    ps = ps_sc.tile([P, DKV], F32, tag="sc", name="ps_vc")
                    for i in range(NCT):
                        nc.tensor.matmul(
                            ps, xq_sb[i][:, tl * P:(tl + 1) * P], wvt_sb[i],
                            start=(i == 0), stop=(i == NCT - 1))
                    nc.scalar.activation(vcloc, ps, AF.Copy)
                    nc.sync.dma_start(out=vc_slice[tl * P:(tl + 1) * P, :], in_=vcloc)

                nc.gpsimd.collective_compute(
                    "AllGather", mybir.AluOpType.bypass, replica_groups=GROUPS,
                    ins=[kc_slice], outs=[kc_ag])
                nc.gpsimd.collective_compute(
                    "AllGather", mybir.AluOpType.bypass, replica_groups=GROUPS,
                    ins=[vc_slice], outs=[vc_ag])

                # Load the gathered K/V back into SBUF.
                for j in range(2):
                    for r in range(CHUNKS):
                        nc.sync.dma_start(
                            out=kt_sb[j][:, r * SQ:(r + 1) * SQ],
                            in_=kc_ag[r, j * P:(j + 1) * P, :])
                for t in range(NT):
                    nc.sync.dma_start(
                        out=vc_sb[t], in_=vc_ag[t // 8, (t % 8) * P:(t % 8 + 1) * P, :])

            # Q^T for the local queries (overlaps the collective when use_cc).
            for j in range(NCT):
                for sh in range(NSH):
                    ps = ps_sc.tile([P, FD], F32, tag="sc", name="ps_qt")
                    for i in range(NCT):
                        nc.tensor.matmul(
                            ps, wqt_sb[i][:, j * P:(j + 1) * P],
                            xq_sb[i][:, sh * FD:(sh + 1) * FD],
                            start=(i == 0), stop=(i == NCT - 1))
                    nc.scalar.activation(qt_sb[j][:, sh * FD:(sh + 1) * FD], ps, AF.Copy)

            if not use_cc:
                # -- recompute full-batch K/V on every core from xt --
                for j in range(2):
                    for tch in range(S // FD):
                        ps = ps_sc.tile([P, FD], F32, tag="sc", name="ps_kt")
                        for i in range(NCT):
                            nc.tensor.matmul(
                                ps, wkt_sb[i][:, j * P:(j + 1) * P],
                                xt_sb[i][:, tch * FD:(tch + 1) * FD],
                                start=(i == 0), stop=(i == NCT - 1))
                        nc.scalar.activation(kt_sb[j][:, tch * FD:(tch + 1) * FD], ps, AF.Copy)
                for t in range(NT):
                    ps = ps_sc.tile([P, DKV], F32, tag="sc", name="ps_vc")
                    for i in range(NCT):
                        nc.tensor.matmul(
                            ps, xt_sb[i][:, t * P:(t + 1) * P], wvt_sb[i],
                            start=(i == 0), stop=(i == NCT - 1))
                    nc.scalar.activation(vc_sb[t], ps, AF.Copy)

        # ---- attention phase (new pools reuse loadin's SBUF) ----
        attp = ctx.enter_context(tc.tile_pool(name="attp", bufs=1))
        pt_pool = ctx.enter_context(tc.tile_pool(name="pt", bufs=NT))
        lpool = ctx.enter_context(tc.tile_pool(name="lpool", bufs=2))
        ypool = ctx.enter_context(tc.tile_pool(name="ypool", bufs=2))

        wot_sb = [attp.tile([P, DM], BF16, tag=f"wo{j}", name=f"wot_sb{j}") for j in range(NCT)]
        for j in range(NCT):
            nc.sync.dma_start(out=wot_sb[j], in_=wot[j * P:(j + 1) * P, :])

        for h in range(H):
            # scores^T + exp + row-sum partials
            lpart = lpool.tile([P, SQ], F32, tag="lp", name="lpart")
            pt_tiles = []
            for t in range(NT):
                ptt = pt_pool.tile([P, SQ], BF16, tag="pt", name="pt_t")
                pt_tiles.append(ptt)
                if EXP_WIDE:
                    ps = ps_sc.tile([P, SQ], F32, tag="sc", name="ps_s")
                    for sh in range(NSH):
                        nc.tensor.matmul(
                            ps[:, sh * FD:(sh + 1) * FD],
                            kt_sb[0][:, t * P:(t + 1) * P],
                            qt_sb[2 * h][:, sh * FD:(sh + 1) * FD],
                            start=True, stop=False)
                        nc.tensor.matmul(
                            ps[:, sh * FD:(sh + 1) * FD],
                            kt_sb[1][:, t * P:(t + 1) * P],
                            qt_sb[2 * h + 1][:, sh * FD:(sh + 1) * FD],
                            start=False, stop=True)
                    nc.scalar.activation(ptt, ps, AF.Exp, scale=SCALE)
                else:
                    for sh in range(NSH):
                        ps = ps_sc.tile([P, FD], F32, tag="sc", name="ps_s")
                        nc.tensor.matmul(
                            ps, kt_sb[0][:, t * P:(t + 1) * P],
                            qt_sb[2 * h][:, sh * FD:(sh + 1) * FD],
                            start=True, stop=False)
                        nc.tensor.matmul(
                            ps, kt_sb[1][:, t * P:(t + 1) * P],
                            qt_sb[2 * h + 1][:, sh * FD:(sh + 1) * FD],
                            start=False, stop=True)
                        nc.scalar.activation(
                            ptt[:, sh * FD:(sh + 1) * FD], ps, AF.Exp, scale=SCALE)
                if t == 0:
                    nc.vector.tensor_copy(lpart, ptt)
                else:
                    nc.vector.tensor_add(lpart, lpart, ptt)

            # unnormalised attention output: O~^T[d, s] += Vc[t,d]^T P^T[t,s]
            if ATTNV_INTERLEAVE:
                # All four (sh, d-half) accumulators run in one t loop so
                # each PT tile is fully consumed at iteration t.
                ov_pairs = [
                    (ps_ov.tile([P, FD], F32, tag="ov", name="ps_ov0"),
                     ps_ov.tile([P, FD], F32, tag="ov", name="ps_ov1"))
                    for _ in range(NSH)
                ]
                for t in range(NT):
                    for dh in range(2):
                        for sh in range(NSH):
                            nc.tensor.matmul(
                                ov_pairs[sh][dh], vc_sb[t][:, dh * P:(dh + 1) * P],
                                pt_tiles[t][:, sh * FD:(sh + 1) * FD],
                                start=(t == 0), stop=(t == NT - 1))
            else:
                # One (sh) pair at a time: 2 live accumulators, 4 bufs ->
                # the pool double-buffers across s-chunks and heads.
                ov_pairs = []
                for sh in range(NSH):
                    ov0 = ps_ov.tile([P, FD], F32, tag="ov", name="ps_ov0")
                    ov1 = ps_ov.tile([P, FD], F32, tag="ov", name="ps_ov1")
                    ov_pairs.append((ov0, ov1))
                    for t in range(NT):
                        nc.tensor.matmul(
                            ov0, vc_sb[t][:, 0:P],
                            pt_tiles[t][:, sh * FD:(sh + 1) * FD],
                            start=(t == 0), stop=(t == NT - 1))
                        nc.tensor.matmul(
                            ov1, vc_sb[t][:, P:DKV],
                            pt_tiles[t][:, sh * FD:(sh + 1) * FD],
                            start=(t == 0), stop=(t == NT - 1))

            # l = sum_t P^T[t, s] (partition sum via ones matmul), rl = 1/l,
            # RL = broadcast of rl over 128 partitions (rank-1 matmul).
            rlb = lpool.tile([P, SQ], F32, tag="rlb", name="rlb")
            if LRL_SPLIT:
                # one-bank l/RL pipeline, processed per s-chunk
                rl_row = lpool.tile([1, SQ], F32, tag="rl_row", name="rl_row")
                for sh in range(NSH):
                    l_ps = ps_lrl.tile([1, FD], F32, tag="lrl", name="ps_l")
                    nc.tensor.matmul(
                        l_ps, ones_col, lpart[:, sh * FD:(sh + 1) * FD],
                        start=True, stop=True)
                    nc.vector.reciprocal(rl_row[:, sh * FD:(sh + 1) * FD], l_ps)
                    rl_ps = ps_lrl.tile([P, FD], F32, tag="lrl", name="ps_rl")
                    nc.tensor.matmul(
                        rl_ps, ones_row, rl_row[:, sh * FD:(sh + 1) * FD],
                        start=True, stop=True)
                    nc.scalar.activation(rlb[:, sh * FD:(sh + 1) * FD], rl_ps, AF.Copy)
            else:
                l_ps = ps_lrl.tile([1, SQ], F32, tag="lrl", name="ps_l")
                for sh in range(NSH):
                    nc.tensor.matmul(
                        l_ps[:, sh * FD:(sh + 1) * FD], ones_col,
                        lpart[:, sh * FD:(sh + 1) * FD], start=True, stop=True)
                rl_row = lpool.tile([1, SQ], F32, tag="rl_row", name="rl_row")
                nc.vector.reciprocal(rl_row, l_ps)
                rl_ps = ps_lrl.tile([P, SQ], F32, tag="lrl", name="ps_rl")
                for sh in range(NSH):
                    nc.tensor.matmul(
                        rl_ps[:, sh * FD:(sh + 1) * FD], ones_row,
                        rl_row[:, sh * FD:(sh + 1) * FD], start=True, stop=True)
                nc.scalar.activation(rlb, rl_ps, AF.Copy)

            # normalise while copying PSUM -> SBUF (bf16 for the W_o matmul)
            for sh in range(NSH):
                ov0, ov1 = ov_pairs[sh]
                nc.vector.tensor_mul(
                    ot_sb[2 * h][:, sh * FD:(sh + 1) * FD], ov0,
                    rlb[:, sh * FD:(sh + 1) * FD])
                nc.vector.tensor_mul(
                    ot_sb[2 * h + 1][:, sh * FD:(sh + 1) * FD], ov1,
                    rlb[:, sh * FD:(sh + 1) * FD])

        # ---- phase W: Y = O @ W_o^T ----
        for sb in range(SQ // P):
            ysb = ypool.tile([P, DM], F32, tag="y", name="ysb")
            for ec in range(DM // FD):
                ps = ps_sc.tile([P, FD], F32, tag="sc", name="ps_y")
                for j in range(NCT):
                    nc.tensor.matmul(
                        ps, ot_sb[j][:, sb * P:(sb + 1) * P],
                        wot_sb[j][:, ec * FD:(ec + 1) * FD],
                        start=(j == 0), stop=(j == NCT - 1))
                nc.scalar.activation(ysb[:, ec * FD:(ec + 1) * FD], ps, AF.Copy)
            nc.sync.dma_start(out=y[sb * P:(sb + 1) * P, :], in_=ysb)


_BUILD_CACHE = {}


def build_program(use_cc=USE_CC):
    """Build + compile the per-core Bass program (cached per process)."""
    key = ("nc", use_cc)
    if key in _BUILD_CACHE:
        return _BUILD_CACHE[key]
    nc = bacc.Bacc("TRN2", target_bir_lowering=False, debug=False,
                   num_devices=NCORES)
    xt = (nc.dram_tensor("xt", [DM, S], BF16, kind="ExternalInput").ap()
          if not use_cc else None)
    xq = nc.dram_tensor("xq", [DM, SQ], BF16, kind="ExternalInput").ap()
    wqt = nc.dram_tensor("wqt", [DM, DM], BF16, kind="ExternalInput").ap()
    wkt = nc.dram_tensor("wkt", [DM, DKV], BF16, kind="ExternalInput").ap()
    wvt = nc.dram_tensor("wvt", [DM, DKV], BF16, kind="ExternalInput").ap()
    wot = nc.dram_tensor("wot", [DM, DM], BF16, kind="ExternalInput").ap()
    y = nc.dram_tensor("y", [SQ, DM], F32, kind="ExternalOutput").ap()
    with tile.TileContext(nc) as tc:
        _emit_full(tc, (y, xt, xq, wqt, wkt, wvt, wot), use_cc)
    nc.compile()
    _BUILD_CACHE[key] = nc
    return nc


def make_in_maps(X, W_q, W_k, W_v, W_o, use_cc=USE_CC):
    """Host-side shard prep: transpose + bf16-cast, one input dict per core."""
    wqt = np.ascontiguousarray(W_q.T).astype(NPBF16)
    wkt = np.ascontiguousarray(W_k.T).astype(NPBF16)
    wvt = np.ascontiguousarray(W_v.T).astype(NPBF16)
    wot = np.ascontiguousarray(W_o.T).astype(NPBF16)
    xts = [np.ascontiguousarray(X[b].T).astype(NPBF16) for b in range(B)]
    in_maps = []
    for c in range(NCORES):
        b, chunk = divmod(c, CHUNKS)
        xq = np.ascontiguousarray(xts[b][:, chunk * SQ:(chunk + 1) * SQ])
        m = {"xq": xq, "wqt": wqt, "wkt": wkt, "wvt": wvt, "wot": wot}
        if not use_cc:
            m["xt"] = xts[b]
        in_maps.append(m)
    return in_maps


def run(X, W_q, W_k, W_v, W_o, trace=False, trace_cores=None, use_cc=USE_CC):
    """Run the 8-core kernel; returns (Y, BassKernelResults)."""
    nc = build_program(use_cc)
    in_maps = make_in_maps(X, W_q, W_k, W_v, W_o, use_cc)
    res = run_bass_kernel_spmd(
        nc, in_maps, list(range(NCORES)), trace=trace, trace_cores=trace_cores)
    Y = np.empty((B, S, DM), np.float32)
    for c in range(NCORES):
        b, chunk = divmod(c, CHUNKS)
        Y[b, chunk * SQ:(chunk + 1) * SQ, :] = res.results[c]["y"]
    return Y, res


def kernel(X, W_q, W_k, W_v, W_o):
    X = np.asarray(X)
    W_q = np.asarray(W_q)
    W_k = np.asarray(W_k)
    W_v = np.asarray(W_v)
    W_o = np.asarray(W_o)
    Y, _ = run(X, W_q, W_k, W_v, W_o)
    return Y


# revision 17
# speedup vs baseline: 28.6587x; 23.6708x over previous
"""Multi-head latent attention (MLA) on Trainium2 — 8-core SPMD Bass kernel.

Reference computation (fp32):
    Q  = X @ W_q.T           [B,S,1024] -> heads [B,H,S,256]
    Kc = X @ W_k.T           [B,S,256]  (shared across heads, MQA-style)
    Vc = X @ W_v.T           [B,S,256]
    P  = softmax(Q Kc^T / sqrt(256))
    Y  = concat_h(P Vc) @ W_o.T

Sharding: 8 cores = (batch b in {0,1}) x (query s-chunk in {0..3}).
Each core projects Q for its own 1024-token chunk, recomputes the (small,
shared) compressed Kc^T / Vc for the whole batch from X_b^T, runs attention
for all 4 heads over its queries, and writes its [1024, 1024] fp32 output
slice.  Host concatenates.  (An AllGather variant that shares Kc/Vc across
cores exists behind use_cc=True, but intra-chip collectives crash the
NRT runtime on this stack and the cost model predicts they lose anyway.)

All matmuls run in bf16 with fp32 PSUM accumulation; softmax runs in fp32 on
the scalar (ACT) engine.  Scores are computed transposed (keys on partitions)
so softmax-normalisation is deferred: the P^T @ Vc matmuls are unnormalised
and each head's output is scaled by 1/l (broadcast via a rank-1 matmul)
before the W_o projection.  Row sums l accumulate on the idle vector engine.

Measured numerics (CoreSim + HW): rel-fro err ~4.3e-3 vs fp32 reference.
"""

import numpy as np
import ml_dtypes
from contextlib import ExitStack

import concourse.bass as bass
import concourse.tile as tile
from concourse import bacc, mybir
from concourse.bass_utils import run_bass_kernel_spmd

# ---- problem constants (hardcoded; kernel.py must be self-contained) ----
B, S, DM = 2, 4096, 1024
H, DK, DKV = 4, 256, 256
NCORES = 8
CHUNKS = 4                # query chunks per batch
SQ = S // CHUNKS          # 1024 queries per core
SCALE = 1.0 / 16.0        # 1/sqrt(DK)

P = 128                   # partitions
NT = S // P               # 32 key tiles
NCT = DM // P             # 8 contraction tiles over the model dim
FD = 512                  # matmul moving free-dim chunk (one fp32 PSUM bank)
NSH = SQ // FD            # 2 query free-dim chunks

BF16 = mybir.dt.bfloat16
F32 = mybir.dt.float32
NPBF16 = ml_dtypes.bfloat16

# Use on-device AllGather to share Kc/Vc across the 4 cores of a batch
# (False recomputes them from the full X_b on every core).  AllGather is
# predicted slower by the cost model AND crashes NRT on this stack -> False.
USE_CC = False

# Tuned emission knobs (cost-model swept): PSUM banks 4+3+1 = 8.
ATTNV_INTERLEAVE = False
PS_SC_BUFS = 4     # scores/projection/W_o accumulators ([128,512] fp32 banks)
PS_OV_BUFS = 3     # attn@V accumulators
PS_LRL_BUFS = 1    # l / 1-l broadcast pipeline (one bank, per-chunk)
LRL_SPLIT = True
EXP_WIDE = False


def _emit_full(tc: tile.TileContext, nc_io, use_cc, rep=0):
    """Emit the complete per-core program (projections + attention + W_o)."""
    nc = tc.nc
    AF = mybir.ActivationFunctionType
    y, xt, xq, wqt, wkt, wvt, wot = nc_io

    with ExitStack() as ctx:
        acts = ctx.enter_context(tc.tile_pool(name=f"acts{rep}", bufs=1))

        ps_sc = ctx.enter_context(tc.tile_pool(name=f"ps_sc{rep}", bufs=PS_SC_BUFS, space="PSUM"))
        ps_ov = ctx.enter_context(tc.tile_pool(name=f"ps_ov{rep}", bufs=PS_OV_BUFS, space="PSUM"))
        ps_lrl = ctx.enter_context(tc.tile_pool(name=f"ps_lrl{rep}", bufs=PS_LRL_BUFS, space="PSUM"))

        qt_sb = [acts.tile([P, SQ], BF16, tag=f"qt{j}", name=f"qt_sb{j}") for j in range(NCT)]
        kt_sb = [acts.tile([P, S], BF16, tag=f"kt{j}", name=f"kt_sb{j}") for j in range(2)]
        vc_sb = [acts.tile([P, DKV], BF16, tag=f"vc{t}", name=f"vc_sb{t}") for t in range(NT)]
        ot_sb = [acts.tile([P, SQ], BF16, tag=f"ot{j}", name=f"ot_sb{j}") for j in range(NCT)]
        ones_col = acts.tile([P, 1], F32, tag="ones_col", name="ones_col")
        ones_row = acts.tile([1, P], F32, tag="ones_row", name="ones_row")
        nc.vector.memset(ones_col, 1.0)
        nc.vector.memset(ones_row, 1.0)

        # ---- phase P (projection inputs live only inside this block) ----
        with tc.tile_pool(name=f"loadin{rep}", bufs=1) as loadin:
            xq_sb = [loadin.tile([P, SQ], BF16, tag=f"xq{i}", name=f"xq_sb{i}") for i in range(NCT)]
            wqt_sb = [loadin.tile([P, DM], BF16, tag=f"wq{i}", name=f"wqt_sb{i}") for i in range(NCT)]
            wkt_sb = [loadin.tile([P, DKV], BF16, tag=f"wk{i}", name=f"wkt_sb{i}") for i in range(NCT)]
            wvt_sb = [loadin.tile([P, DKV], BF16, tag=f"wv{i}", name=f"wvt_sb{i}") for i in range(NCT)]
            for i in range(NCT):
                nc.sync.dma_start(out=xq_sb[i], in_=xq[i * P:(i + 1) * P, :])
                nc.sync.dma_start(out=wqt_sb[i], in_=wqt[i * P:(i + 1) * P, :])
            if not use_cc:
                xt_sb = [loadin.tile([P, S], BF16, tag=f"xt{i}", name=f"xt_sb{i}")
                         for i in range(NCT)]
                for i in range(NCT):
                    nc.sync.dma_start(out=xt_sb[i], in_=xt[i * P:(i + 1) * P, :])
                    nc.sync.dma_start(out=wkt_sb[i], in_=wkt[i * P:(i + 1) * P, :])
                    nc.sync.dma_start(out=wvt_sb[i], in_=wvt[i * P:(i + 1) * P, :])
            else:
                for i in range(NCT):
                    nc.sync.dma_start(out=wkt_sb[i], in_=wkt[i * P:(i + 1) * P, :])
                    nc.sync.dma_start(out=wvt_sb[i], in_=wvt[i * P:(i + 1) * P, :])

            if use_cc:
                # -- K/V for the local chunk only, then AllGather over the batch --
                GROUPS = [[0, 1, 2, 3], [4, 5, 6, 7]]
                kc_slice = nc.dram_tensor(f"kc_slice{rep}", [DKV, SQ], BF16).ap()
                vc_slice = nc.dram_tensor(f"vc_slice{rep}", [SQ, DKV], BF16).ap()
                kc_ag = nc.dram_tensor(f"kc_ag{rep}", [CHUNKS, DKV, SQ], BF16).ap()
                vc_ag = nc.dram_tensor(f"vc_ag{rep}", [CHUNKS, SQ, DKV], BF16).ap()

                # Kc^T slice [DKV, SQ] from the local chunk columns (xq).
                for j in range(2):
                    ktloc = loadin.tile([P, SQ], BF16, tag=f"ktloc{j}", name=f"ktloc{j}")
                    for sh in range(NSH):
                        ps = ps_sc.tile([P, FD], F32, tag="sc", name="ps_kt")
                        for i in range(NCT):
                            nc.tensor.matmul(
                                ps, wkt_sb[i][:, j * P:(j + 1) * P],
                                xq_sb[i][:, sh * FD:(sh + 1) * FD],
                                start=(i == 0), stop=(i == NCT - 1))
                        nc.scalar.activation(ktloc[:, sh * FD:(sh + 1) * FD], ps, AF.Copy)
                    nc.sync.dma_start(out=kc_slice[j * P:(j + 1) * P, :], in_=ktloc)

                # Vc slice [SQ, DKV] from the local chunk.
                for tl in range(SQ // P):
                    vcloc = loadin.tile([P, DKV], BF16, tag="vcloc", name="vcloc", bufs=4)
                    ps = ps_sc.tile([P, DKV], F32, tag="sc", name="ps_vc")
                    for i in range(NCT):
                        nc.tensor.matmul(
                            ps, xq_sb[i][:, tl * P:(tl + 1) * P], wvt_sb[i],
                            start=(i == 0), stop=(i == NCT - 1))
                    nc.scalar.activation(vcloc, ps, AF.Copy)
                    nc.sync.dma_start(out=vc_slice[tl * P:(tl + 1) * P, :], in_=vcloc)

                nc.gpsimd.collective_compute(
                    "AllGather", mybir.AluOpType.bypass, replica_groups=GROUPS,
                    ins=[kc_slice], outs=[kc_ag])
                nc.gpsimd.collective_compute(
                    "AllGather", mybir.AluOpType.bypass, replica_groups=GROUPS,
                    ins=[vc_slice], outs=[vc_ag])

                # Load the gathered K/V back into SBUF.
                for j in range(2):
                    for r in range(CHUNKS):
                        nc.sync.dma_start(
                            out=kt_sb[j][:, r * SQ:(r + 1) * SQ],
                            in_=kc_ag[r, j * P:(j + 1) * P, :])
                for t in range(NT):
                    nc.sync.dma_start(
                        out=vc_sb[t], in_=vc_ag[t // 8, (t % 8) * P:(t % 8 + 1) * P, :])

            # Q^T for the local queries (overlaps the collective when use_cc).
            for j in range(NCT):
                for sh in range(NSH):
                    ps = ps_sc.tile([P, FD], F32, tag="sc", name="ps_qt")
                    for i in range(NCT):
                        nc.tensor.matmul(
                            ps, wqt_sb[i][:, j * P:(j + 1) * P],
                            xq_sb[i][:, sh * FD:(sh + 1) * FD],
                            start=(i == 0), stop=(i == NCT - 1))
                    nc.scalar.activation(qt_sb[j][:, sh * FD:(sh + 1) * FD], ps, AF.Copy)

            if not use_cc:
                # -- recompute full-batch K/V on every core from xt --
                for j in range(2):
                    for tch in range(S // FD):
                        ps = ps_sc.tile([P, FD], F32, tag="sc", name="ps_kt")
                        for i in range(NCT):
                            nc.tensor.matmul(
                                ps, wkt_sb[i][:, j * P:(j + 1) * P],
                                xt_sb[i][:, tch * FD:(tch + 1) * FD],
                                start=(i == 0), stop=(i == NCT - 1))
                        nc.scalar.activation(kt_sb[j][:, tch * FD:(tch + 1) * FD], ps, AF.Copy)
                for t in range(NT):
                    ps = ps_sc.tile([P, DKV], F32, tag="sc", name="ps_vc")
                    for i in range(NCT):
                        nc.tensor.matmul(
                            ps, xt_sb[i][:, t * P:(t + 1) * P], wvt_sb[i],
                            start=(i == 0), stop=(i == NCT - 1))
                    nc.scalar.activation(vc_sb[t], ps, AF.Copy)

        # ---- attention phase (new pools reuse loadin's SBUF) ----
        attp = ctx.enter_context(tc.tile_pool(name=f"attp{rep}", bufs=1))
        pt_pool = ctx.enter_context(tc.tile_pool(name=f"pt{rep}", bufs=NT))
        lpool = ctx.enter_context(tc.tile_pool(name=f"lpool{rep}", bufs=2))
        ypool = ctx.enter_context(tc.tile_pool(name=f"ypool{rep}", bufs=2))

        wot_sb = [attp.tile([P, DM], BF16, tag=f"wo{j}", name=f"wot_sb{j}") for j in range(NCT)]
        for j in range(NCT):
            nc.sync.dma_start(out=wot_sb[j], in_=wot[j * P:(j + 1) * P, :])

        for h in range(H):
            # scores^T + exp + row-sum partials
            lpart = lpool.tile([P, SQ], F32, tag="lp", name="lpart")
            pt_tiles = []
            for t in range(NT):
                ptt = pt_pool.tile([P, SQ], BF16, tag="pt", name="pt_t")
                pt_tiles.append(ptt)
                if EXP_WIDE:
                    ps = ps_sc.tile([P, SQ], F32, tag="sc", name="ps_s")
                    for sh in range(NSH):
                        nc.tensor.matmul(
                            ps[:, sh * FD:(sh + 1) * FD],
                            kt_sb[0][:, t * P:(t + 1) * P],
                            qt_sb[2 * h][:, sh * FD:(sh + 1) * FD],
                            start=True, stop=False)
                        nc.tensor.matmul(
                            ps[:, sh * FD:(sh + 1) * FD],
                            kt_sb[1][:, t * P:(t + 1) * P],
                            qt_sb[2 * h + 1][:, sh * FD:(sh + 1) * FD],
                            start=False, stop=True)
                    nc.scalar.activation(ptt, ps, AF.Exp, scale=SCALE)
                else:
                    for sh in range(NSH):
                        ps = ps_sc.tile([P, FD], F32, tag="sc", name="ps_s")
                        nc.tensor.matmul(
                            ps, kt_sb[0][:, t * P:(t + 1) * P],
                            qt_sb[2 * h][:, sh * FD:(sh + 1) * FD],
                            start=True, stop=False)
                        nc.tensor.matmul(
                            ps, kt_sb[1][:, t * P:(t + 1) * P],
                            qt_sb[2 * h + 1][:, sh * FD:(sh + 1) * FD],
                            start=False, stop=True)
                        nc.scalar.activation(
                            ptt[:, sh * FD:(sh + 1) * FD], ps, AF.Exp, scale=SCALE)
                if t == 0:
                    nc.vector.tensor_copy(lpart, ptt)
                else:
                    nc.vector.tensor_add(lpart, lpart, ptt)

            # unnormalised attention output: O~^T[d, s] += Vc[t,d]^T P^T[t,s]
            if ATTNV_INTERLEAVE:
                # All four (sh, d-half) accumulators run in one t loop so
                # each PT tile is fully consumed at iteration t.
                ov_pairs = [
                    (ps_ov.tile([P, FD], F32, tag="ov", name="ps_ov0"),
                     ps_ov.tile([P, FD], F32, tag="ov", name="ps_ov1"))
                    for _ in range(NSH)
                ]
                for t in range(NT):
                    for dh in range(2):
                        for sh in range(NSH):
                            nc.tensor.matmul(
                                ov_pairs[sh][dh], vc_sb[t][:, dh * P:(dh + 1) * P],
                                pt_tiles[t][:, sh * FD:(sh + 1) * FD],
                                start=(t == 0), stop=(t == NT - 1))
            else:
                # One (sh) pair at a time: 2 live accumulators, 4 bufs ->
                # the pool double-buffers across s-chunks and heads.
                ov_pairs = []
                for sh in range(NSH):
                    ov0 = ps_ov.tile([P, FD], F32, tag="ov", name="ps_ov0")
                    ov1 = ps_ov.tile([P, FD], F32, tag="ov", name="ps_ov1")
                    ov_pairs.append((ov0, ov1))
                    for t in range(NT):
                        nc.tensor.matmul(
                            ov0, vc_sb[t][:, 0:P],
                            pt_tiles[t][:, sh * FD:(sh + 1) * FD],
                            start=(t == 0), stop=(t == NT - 1))
                        nc.tensor.matmul(
                            ov1, vc_sb[t][:, P:DKV],
                            pt_tiles[t][:, sh * FD:(sh + 1) * FD],
                            start=(t == 0), stop=(t == NT - 1))

            # l = sum_t P^T[t, s] (partition sum via ones matmul), rl = 1/l,
            # RL = broadcast of rl over 128 partitions (rank-1 matmul).
            rlb = lpool.tile([P, SQ], F32, tag="rlb", name="rlb")
            if LRL_SPLIT:
                # one-bank l/RL pipeline, processed per s-chunk
                rl_row = lpool.tile([1, SQ], F32, tag="rl_row", name="rl_row")
                for sh in range(NSH):
                    l_ps = ps_lrl.tile([1, FD], F32, tag="lrl", name="ps_l")
                    nc.tensor.matmul(
                        l_ps, ones_col, lpart[:, sh * FD:(sh + 1) * FD],
                        start=True, stop=True)
                    nc.vector.reciprocal(rl_row[:, sh * FD:(sh + 1) * FD], l_ps)
                    rl_ps = ps_lrl.tile([P, FD], F32, tag="lrl", name="ps_rl")
                    nc.tensor.matmul(
                        rl_ps, ones_row, rl_row[:, sh * FD:(sh + 1) * FD],
                        start=True, stop=True)
                    nc.scalar.activation(rlb[:, sh * FD:(sh + 1) * FD], rl_ps, AF.Copy)
            else:
                l_ps = ps_lrl.tile([1, SQ], F32, tag="lrl", name="ps_l")
                for sh in range(NSH):
                    nc.tensor.matmul(
                        l_ps[:, sh * FD:(sh + 1) * FD], ones_col,
                        lpart[:, sh * FD:(sh + 1) * FD], start=True, stop=True)
                rl_row = lpool.tile([1, SQ], F32, tag="rl_row", name="rl_row")
                nc.vector.reciprocal(rl_row, l_ps)
                rl_ps = ps_lrl.tile([P, SQ], F32, tag="lrl", name="ps_rl")
                for sh in range(NSH):
                    nc.tensor.matmul(
                        rl_ps[:, sh * FD:(sh + 1) * FD], ones_row,
                        rl_row[:, sh * FD:(sh + 1) * FD], start=True, stop=True)
                nc.scalar.activation(rlb, rl_ps, AF.Copy)

            # normalise while copying PSUM -> SBUF (bf16 for the W_o matmul)
            for sh in range(NSH):
                ov0, ov1 = ov_pairs[sh]
                nc.vector.tensor_mul(
                    ot_sb[2 * h][:, sh * FD:(sh + 1) * FD], ov0,
                    rlb[:, sh * FD:(sh + 1) * FD])
                nc.vector.tensor_mul(
                    ot_sb[2 * h + 1][:, sh * FD:(sh + 1) * FD], ov1,
                    rlb[:, sh * FD:(sh + 1) * FD])

        # ---- phase W: Y = O @ W_o^T ----
        for sb in range(SQ // P):
            ysb = ypool.tile([P, DM], F32, tag="y", name="ysb")
            for ec in range(DM // FD):
                ps = ps_sc.tile([P, FD], F32, tag="sc", name="ps_y")
                for j in range(NCT):
                    nc.tensor.matmul(
                        ps, ot_sb[j][:, sb * P:(sb + 1) * P],
                        wot_sb[j][:, ec * FD:(ec + 1) * FD],
                        start=(j == 0), stop=(j == NCT - 1))
                nc.scalar.activation(ysb[:, ec * FD:(ec + 1) * FD], ps, AF.Copy)
            nc.sync.dma_start(out=y[sb * P:(sb + 1) * P, :], in_=ysb)


_BUILD_CACHE = {}


def build_program(use_cc=USE_CC, reps=1):
    """Build + compile the per-core Bass program (cached per process)."""
    key = ("nc", use_cc, reps)
    if key in _BUILD_CACHE:
        return _BUILD_CACHE[key]
    nc = bacc.Bacc("TRN2", target_bir_lowering=False, debug=False,
                   num_devices=NCORES)
    xt = (nc.dram_tensor("xt", [DM, S], BF16, kind="ExternalInput").ap()
          if not use_cc else None)
    xq = nc.dram_tensor("xq", [DM, SQ], BF16, kind="ExternalInput").ap()
    wqt = nc.dram_tensor("wqt", [DM, DM], BF16, kind="ExternalInput").ap()
    wkt = nc.dram_tensor("wkt", [DM, DKV], BF16, kind="ExternalInput").ap()
    wvt = nc.dram_tensor("wvt", [DM, DKV], BF16, kind="ExternalInput").ap()
    wot = nc.dram_tensor("wot", [DM, DM], BF16, kind="ExternalInput").ap()
    y = nc.dram_tensor("y", [SQ, DM], F32, kind="ExternalOutput").ap()
    with tile.TileContext(nc) as tc:
        for rep in range(reps):
            _emit_full(tc, (y, xt, xq, wqt, wkt, wvt, wot), use_cc, rep=rep)
    nc.compile()
    _BUILD_CACHE[key] = nc
    return nc


def make_in_maps(X, W_q, W_k, W_v, W_o, use_cc=USE_CC):
    """Host-side shard prep: transpose + bf16-cast, one input dict per core."""
    wqt = np.ascontiguousarray(W_q.T).astype(NPBF16)
    wkt = np.ascontiguousarray(W_k.T).astype(NPBF16)
    wvt = np.ascontiguousarray(W_v.T).astype(NPBF16)
    wot = np.ascontiguousarray(W_o.T).astype(NPBF16)
    xts = [np.ascontiguousarray(X[b].T).astype(NPBF16) for b in range(B)]
    in_maps = []
    for c in range(NCORES):
        b, chunk = divmod(c, CHUNKS)
        xq = np.ascontiguousarray(xts[b][:, chunk * SQ:(chunk + 1) * SQ])
        m = {"xq": xq, "wqt": wqt, "wkt": wkt, "wvt": wvt, "wot": wot}
        if not use_cc:
            m["xt"] = xts[b]
        in_maps.append(m)
    return in_maps


def run(X, W_q, W_k, W_v, W_o, trace=False, trace_cores=None, use_cc=USE_CC):
    """Run the 8-core kernel; returns (Y, BassKernelResults)."""
    nc = build_program(use_cc)
    in_maps = make_in_maps(X, W_q, W_k, W_v, W_o, use_cc)
    res = run_bass_kernel_spmd(
        nc, in_maps, list(range(NCORES)), trace=trace, trace_cores=trace_cores)
    Y = np.empty((B, S, DM), np.float32)
    for c in range(NCORES):
        b, chunk = divmod(c, CHUNKS)
        Y[b, chunk * SQ:(chunk + 1) * SQ, :] = res.results[c]["y"]
    return Y, res


def kernel(X, W_q, W_k, W_v, W_o):
    X = np.asarray(X)
    W_q = np.asarray(W_q)
    W_k = np.asarray(W_k)
    W_v = np.asarray(W_v)
    W_o = np.asarray(W_o)
    Y, _ = run(X, W_q, W_k, W_v, W_o)
    return Y
